# revision 42
# baseline (speedup 1.0000x reference)
"""GIN (MoMuGNN) message-passing kernel for 8 TRN2 NeuronCores.

Full inputs in, full output out. All graph compute runs on device:
per-layer edge gather (SWDGE), one-hot scatter-add matmuls into PSUM,
MLP, batch-norm (stats via AllReduce), inter-layer fp16 AllGather of
node features, and the final per-graph segment-max (transpose-gather +
max reduces + AllReduce-max). Host work is limited to data layout
(transpose/cast/shard) and edge-schedule construction, cached per graph.

Edge gathers are descriptor-rate-bound (~8 ns per gathered row on the
GPSIMD SWDGE path), so the schedule minimizes gathered rows: edges are
chunked per (dst-core, 4-window group, src-half) bucket — group-level
chunking needs ~7% padding vs ~16% for per-window chunking — and each
128-edge chunk scatters via one narrow [128x128] one-hot matmul per
128-dst window segment it touches.
"""

import hashlib
import numpy as np
from dataclasses import dataclass, field

import concourse.tile as tile
from concourse import bacc, mybir

P = 128
NC = 8
BN_EPS = 1e-5
F32 = mybir.dt.float32
F16 = mybir.dt.float16


@dataclass
class Cfg:
    N: int
    E: int
    L: int
    G: int
    F: int = 128

    @property
    def npc(self):
        return self.N // NC

    @property
    def half(self):
        return self.N // 2

    @property
    def ntiles(self):
        return (self.npc + P - 1) // P

    def tsize(self, t):
        return min(P, self.npc - t * P)

    @property
    def groups(self):
        gs = []
        t = 0
        while t < self.ntiles:
            gs.append(list(range(t, min(t + 4, self.ntiles))))
            t += 4
        return gs


def _wrap_idx16(flat_idx: np.ndarray, nchunks: int) -> np.ndarray:
    """[nchunks*128] uint16 -> [128, nchunks*8] int16 in the wrapped layout
    dma_gather expects (16-partition wrap, replicated to 128)."""
    w16 = np.zeros((16, nchunks * 8), np.uint16)
    fi = flat_idx.reshape(nchunks * 8, 16)
    w16[:, :] = fi.T
    return np.tile(w16, (8, 1)).view(np.int16)


@dataclass
class Sched:
    K2: np.ndarray         # [ngroups, 2] chunks per (group, src-half), max over cores
    group_chunks: list     # per group: list (consumption order) of seg lists [(w, segcol)]
    nseg: int
    total_chunks: int
    idx16: list            # per core: [128, total_chunks*8] int16 wrapped
    dstl: list             # per core: [128, nseg] fp32 (window-local dst, -1 pad)
    # ---- segment-max mask/select schedule ----
    # per (core, tile): slots = (graph x tile) spans, padded to the per-tile
    # max count S_t over cores. gm_lo holds each tile's S_t+1 ascending span
    # boundaries; gm_sel holds K one-hot slot->graph selection matrices.
    gm_S: list = field(default_factory=list)        # per tile: padded slot count
    gm_lo: list = field(default_factory=list)       # per core: [128, nlocol] f32
    gm_sel: list = field(default_factory=list)      # per core: [128, K*G] f32
    gm_K: int = 0
    gm_nslots: int = 0
    gm_nlocol: int = 0
    degn: list = field(default_factory=list)        # per core: [1, npc] f16, -(deg+1)


def build_schedule(cfg: Cfg, edge_index: np.ndarray, batch: np.ndarray) -> Sched:
    """Bucket edges per (dst-core, 4-window group, src-half); within a bucket
    edges are sorted by (window, src). Chunks are 128 edges; a chunk that
    straddles windows gets one one-hot segment per window (union over cores
    so the SPMD instruction stream is uniform; absent windows get all -1
    dst columns). Also builds the segment-max gather schedule."""
    src = edge_index[0].astype(np.int64)
    dst = edge_index[1].astype(np.int64)
    npc, half, ntiles = cfg.npc, cfg.half, cfg.ntiles
    groups = cfg.groups
    ngroups = len(groups)
    core = dst // npc
    loc = dst % npc
    wi = loc // P                  # window (=tile) within core
    dl = loc - wi * P              # dst local within window
    gi_of_w = np.zeros(ntiles, np.int64)
    for i, g in enumerate(groups):
        for w in g:
            gi_of_w[w] = i
    gidx = gi_of_w[wi]
    # src-half split at tile-aligned local row H0 so AllGather can be issued
    # in two stages and lo gathers start after stage 0
    H0 = 20 * P                     # 2560 local rows -> table0 (stage-0 AG
                                    # issues after group 4, mid-layer)
    sc_ = src // npc
    sr_ = src % npc
    hf = (sr_ >= H0).astype(np.int64)

    order = np.lexsort((src, wi, hf, gidx, core))
    cs = core[order]
    gs_ = gidx[order]
    hs = hf[order]
    ws_ = wi[order]
    tab_row = np.where(hf == 0, sc_ * H0 + sr_,
                       sc_ * (npc - H0) + (sr_ - H0))
    srcs = tab_row[order]
    dls = dl[order]
    key = (cs * ngroups + gs_) * 2 + hs
    bounds = np.searchsorted(key, np.arange(NC * ngroups * 2 + 1))
    buckets = {}
    cnt = np.zeros((NC, ngroups, 2), np.int64)
    for c in range(NC):
        for gi in range(ngroups):
            for h in range(2):
                k = (c * ngroups + gi) * 2 + h
                a, b = bounds[k], bounds[k + 1]
                buckets[(c, gi, h)] = (srcs[a:b], dls[a:b], ws_[a:b])
                cnt[c, gi, h] = b - a

    K2 = np.zeros((ngroups, 2), np.int64)
    for gi in range(ngroups):
        for h in range(2):
            m = cnt[:, gi, h].max()
            K2[gi, h] = (m + P - 1) // P if m > 0 else 0
        if K2[gi].sum() == 0:
            K2[gi, 0] = 1

    # segments: per (group, half, chunk) the union of windows over cores
    group_chunks = [[] for _ in range(ngroups)]
    seg_cols = 0
    chunk_cols = 0
    for gi in range(ngroups):
        for h in range(2):
            for j in range(int(K2[gi, h])):
                wins = set()
                for c in range(NC):
                    _s, _d, wv = buckets[(c, gi, h)]
                    seg = wv[j * P:(j + 1) * P]
                    if len(seg):
                        wins.update(np.unique(seg).tolist())
                if not wins:
                    wins = {groups[gi][0]}
                segs = [(int(w), seg_cols + k) for k, w in enumerate(sorted(wins))]
                seg_cols += len(segs)
                group_chunks[gi].append(segs)
                chunk_cols += 1
    nseg = seg_cols
    total_chunks = chunk_cols

    idx16, dstl = [], []
    for c in range(NC):
        flat_idx = np.zeros(total_chunks * P, np.uint16)
        flat_dl = np.full((P, nseg), -1.0, np.float32)
        pos = 0
        for gi in range(ngroups):
            ci = 0
            for h in range(2):
                s_arr, d_arr, w_arr = buckets[(c, gi, h)]
                n = len(s_arr)
                for j in range(int(K2[gi, h])):
                    rows = slice(j * P, (j + 1) * P)
                    sseg = s_arr[rows]
                    flat_idx[pos * P:pos * P + len(sseg)] = sseg.astype(np.uint16)
                    dseg = d_arr[rows]
                    wseg = w_arr[rows]
                    for (w, scol) in group_chunks[gi][ci]:
                        col = np.full(P, -1.0, np.float32)
                        m = wseg == w
                        col[:len(dseg)][m] = dseg[m].astype(np.float32)
                        flat_dl[:, scol] = col
                    pos += 1
                    ci += 1
        assert pos == total_chunks
        idx16.append(_wrap_idx16(flat_idx, total_chunks))
        dstl.append(flat_dl)

    # ---- segment-max mask/select schedule ---------------------------------
    # Graph spans are contiguous in local node order (batch sorted), so each
    # (graph, tile) incidence is a column span [lo, hi) inside the tile; the
    # spans of a tile's slots partition its columns. The per-tile slot count
    # is padded to S_t = max over cores (zero-width spans); per-core span
    # boundaries flow in as data (gm_lo). Per-graph combine: each graph's
    # slots are consecutive; K one-hot matrices select the k-th slot of each
    # graph (sentinel column 127 for absent/padded picks), max-combined.
    batch = np.asarray(batch, np.int64)
    G = cfg.G
    starts = np.searchsorted(batch, np.arange(G))
    ends = np.searchsorted(batch, np.arange(1, G + 1))
    core_tile_slots = []
    for c in range(NC):
        c0, c1 = c * npc, (c + 1) * npc
        per_tile = [[] for _ in range(ntiles)]
        for g in range(G):
            a, b = max(int(starts[g]), c0), min(int(ends[g]), c1)
            if b <= a:
                continue
            la, lb = a - c0, b - c0
            for t in range(la // P, (lb - 1) // P + 1):
                lo = max(la - t * P, 0)
                hi = min(lb - t * P, cfg.tsize(t))
                per_tile[t].append((g, lo, hi))
        core_tile_slots.append(per_tile)
    gm_S = [max(len(core_tile_slots[c][t]) for c in range(NC))
            for t in range(ntiles)]
    slot_base = np.concatenate([[0], np.cumsum(gm_S)]).astype(int)
    gm_nslots = int(slot_base[-1])
    assert gm_nslots <= 127, gm_nslots
    SENT = 127
    gm_nlocol = int(sum(s + 1 for s in gm_S))
    gm_lo, gslots_all = [], []
    gm_K = 1
    for c in range(NC):
        lo_flat = np.zeros(gm_nlocol, np.float32)
        gslots = {}
        off = 0
        for t in range(ntiles):
            ts_ = float(cfg.tsize(t))
            slots = core_tile_slots[c][t]
            col = np.full(gm_S[t] + 1, ts_, np.float32)
            for i, (g, lo, hi) in enumerate(slots):
                col[i] = float(lo)
                gslots.setdefault(g, []).append(int(slot_base[t]) + i)
            lo_flat[off:off + gm_S[t] + 1] = col
            off += gm_S[t] + 1
        gslots_all.append(gslots)
        if gslots:
            gm_K = max(gm_K, max(len(v) for v in gslots.values()))
        gm_lo.append(np.broadcast_to(lo_flat, (P, gm_nlocol)).copy())
    gm_sel = []
    for c in range(NC):
        sel = np.zeros((P, gm_K * G), np.float32)
        gslots = gslots_all[c]
        for g in range(G):
            sl = gslots.get(g, [])
            for k in range(gm_K):
                s = sl[k] if k < len(sl) else SENT
                sel[s, k * G + g] = 1.0
        gm_sel.append(sel)

    degn = []
    for c in range(NC):
        cnt_c = np.bincount(loc[core == c], minlength=npc).astype(np.float64)
        degn.append((-(cnt_c + 1.0)).astype(np.float16).reshape(1, npc))

    return Sched(K2=K2, group_chunks=group_chunks, nseg=nseg,
                 total_chunks=total_chunks, idx16=idx16, dstl=dstl,
                 gm_S=gm_S, gm_lo=gm_lo, gm_sel=gm_sel, gm_K=gm_K,
                 gm_nslots=gm_nslots, gm_nlocol=gm_nlocol, degn=degn)


def build_nc(cfg: Cfg, sched: Sched, *, no_ar=False, no_ag=False,
             self_only=False, no_segmax=False, gather_only=False):
    npc, ntiles, L, N, G = cfg.npc, cfg.ntiles, cfg.L, cfg.N, cfg.G
    half = cfg.half
    TC = sched.total_chunks
    NSEG = sched.nseg
    K2 = sched.K2
    GMK = sched.gm_K
    SMAX = max(sched.gm_S)
    gm_lo_off = np.concatenate([[0], np.cumsum([s + 1 for s in sched.gm_S])])
    gm_slot_base = np.concatenate([[0], np.cumsum(sched.gm_S)]).astype(int)
    GM_BIG = 4096.0       # |raw z| < ~40, so z+BIG stays positive; fp32 step
    GM_SENT = -30000.0    # at 4096 is 2.4e-4 -- negligible vs the 2e-2 gate
    relu_op = mybir.ActivationFunctionType.Relu
    copy_op = mybir.ActivationFunctionType.Copy
    ident_op = mybir.ActivationFunctionType.Identity

    nc = bacc.Bacc("TRN2", target_bir_lowering=False, debug=False, num_devices=NC,
                   num_swdge_queues=4)

    xh_d = nc.dram_tensor("x_hrm", [P, ntiles * P], F16, kind="ExternalInput")
    xt_d = nc.dram_tensor("x_tab", [N, P], F16, kind="ExternalInput")
    idx_d = nc.dram_tensor("idx16", [P, TC * 8], mybir.dt.int16, kind="ExternalInput")
    gmlo_d = nc.dram_tensor("gmlo", [P, sched.gm_nlocol], F32,
                            kind="ExternalInput")
    gmsel_d = nc.dram_tensor("gmsel", [P, GMK * G], F32, kind="ExternalInput")
    degn_d = nc.dram_tensor("degn", [1, npc], F16, kind="ExternalInput")
    dstl_d = nc.dram_tensor("dstl", [P, NSEG], F32, kind="ExternalInput")
    iota_d = nc.dram_tensor("iota", [P, P], F32, kind="ExternalInput")
    ident_d = nc.dram_tensor("ident", [P, P], F32, kind="ExternalInput")
    w1_d = nc.dram_tensor("w1", [P, L * 2 * P], F32, kind="ExternalInput")
    w2_d = nc.dram_tensor("w2", [P, L * 2 * P], F32, kind="ExternalInput")
    b1_d = nc.dram_tensor("b1", [P, L * 2], F32, kind="ExternalInput")
    b2_d = nc.dram_tensor("b2", [P, L], F32, kind="ExternalInput")
    gam_d = nc.dram_tensor("gam", [P, L], F32, kind="ExternalInput")
    bet_d = nc.dram_tensor("bet", [P, L], F32, kind="ExternalInput")

    gmax_out = nc.dram_tensor("gmaxT", [G, P], F32, kind="ExternalOutput")

    H0 = 20 * P
    H1 = npc - H0
    ag_in0 = [nc.dram_tensor(f"ag_in0_{l}", [H0, P], F16, kind="Internal")
              for l in range(L - 1)]
    ag_in1 = [nc.dram_tensor(f"ag_in1_{l}", [H1, P], F16, kind="Internal")
              for l in range(L - 1)]
    ag_out0 = [nc.dram_tensor(f"ag_out0_{l}", [NC * H0, P], F16,
                              kind="Internal", addr_space="Shared")
               for l in range(L - 1)]
    ag_out1 = [nc.dram_tensor(f"ag_out1_{l}", [NC * H1, P], F16,
                              kind="Internal", addr_space="Shared")
               for l in range(L - 1)]
    ar_in = [nc.dram_tensor(f"ar_in_{l}", [P, 2], F32, kind="Internal")
             for l in range(L)]
    ar_out = [nc.dram_tensor(f"ar_out_{l}", [P, 2], F32, kind="Internal",
                             addr_space="Shared") for l in range(L)]
    gm_in = nc.dram_tensor("gm_in", [G, P], F32, kind="Internal")
    gm_out = nc.dram_tensor("gm_out", [G, P], F32, kind="Internal",
                            addr_space="Shared")
    rg = [list(range(NC))]

    inv_n = 1.0 / N

    with tile.TileContext(nc) as tc:
        with tc.tile_pool(name="const", bufs=1) as cp, \
             tc.tile_pool(name="gath", bufs=5) as gp, \
             tc.tile_pool(name="oh", bufs=2) as ohp, \
             tc.tile_pool(name="zn", bufs=3) as znp, \
             tc.tile_pool(name="u", bufs=2) as up, \
             tc.tile_pool(name="small", bufs=8) as sp, \
             tc.tile_pool(name="scr", bufs=2) as scrp, \
             tc.tile_pool(name="mk", bufs=2) as mkp, \
             tc.tile_pool(name="ps_agg", bufs=2, space="PSUM") as pagg, \
             tc.tile_pool(name="ps_mlp", bufs=2, space="PSUM") as pmlp, \
             tc.tile_pool(name="ps_tp", bufs=2, space="PSUM") as ptp:

            # ---- persistent SBUF ----
            idx_sb = cp.tile([P, TC * 8], mybir.dt.int16)
            nc.sync.dma_start(out=idx_sb[:], in_=idx_d[:, :])
            gmlo_sb = cp.tile([P, sched.gm_nlocol], F32)
            nc.sync.dma_start(out=gmlo_sb[:], in_=gmlo_d[:, :])
            gmsel_sb = cp.tile([P, GMK * G], F32)
            nc.sync.dma_start(out=gmsel_sb[:], in_=gmsel_d[:, :])
            dstl_sb = cp.tile([P, NSEG], F32)
            nc.sync.dma_start(out=dstl_sb[:], in_=dstl_d[:, :])
            iota_sb = cp.tile([P, P], F32)
            nc.sync.dma_start(out=iota_sb[:], in_=iota_d[:, :])
            ident_sb = cp.tile([P, P], F32)
            nc.sync.dma_start(out=ident_sb[:], in_=ident_d[:, :])
            w1_sb = cp.tile([P, L * 2 * P], F32)
            nc.sync.dma_start(out=w1_sb[:], in_=w1_d[:, :])
            w2_sb = cp.tile([P, L * 2 * P], F32)
            nc.sync.dma_start(out=w2_sb[:], in_=w2_d[:, :])
            b1_sb = cp.tile([P, L * 2], F32)
            nc.sync.dma_start(out=b1_sb[:], in_=b1_d[:, :])
            b2_sb = cp.tile([P, L], F32)
            nc.sync.dma_start(out=b2_sb[:], in_=b2_d[:, :])
            gam_sb = cp.tile([P, L], F32)
            nc.sync.dma_start(out=gam_sb[:], in_=gam_d[:, :])
            bet_sb = cp.tile([P, L], F32)
            nc.sync.dma_start(out=bet_sb[:], in_=bet_d[:, :])

            eps_sb = cp.tile([P, 1], F32)
            nc.vector.memset(eps_sb[:], BN_EPS)
            zero_sb = cp.tile([P, 1], F32)
            nc.vector.memset(zero_sb[:], 0.0)
            big_sb = cp.tile([P, 1], F32)
            nc.vector.memset(big_sb[:], GM_BIG)
            gmM = cp.tile([P, P], F32)
            nc.vector.memset(gmM[:], 0.0)
            iota16 = cp.tile([P, P], F16)
            nc.vector.tensor_copy(out=iota16[:], in_=iota_sb[:])
            ident16 = cp.tile([P, P], F16)
            nc.vector.tensor_copy(out=ident16[:], in_=ident_sb[:])
            dstl16 = cp.tile([P, NSEG], F16)
            nc.vector.tensor_copy(out=dstl16[:], in_=dstl_sb[:])
            hrm = [cp.tile([P, ntiles * P], F16, name=f"hrm{i}") for i in range(2)]
            nc.sync.dma_start(out=hrm[1][:], in_=xh_d[:, :])
            nc.vector.memset(hrm[0][:], 0.0)
            scol_hist = cp.tile([P, L], F32)
            urep = [cp.tile([P, P], F16, name=f"urep{i}") for i in range(2)]
            urow = [cp.tile([P, P], F16, name=f"urow{i}") for i in range(2)]
            ones_row = cp.tile([1, P], F16, name="ones_row")
            nc.vector.memset(ones_row[:, :], 1.0)
            degn_sb = cp.tile([1, npc], F16, name="degn")
            nc.sync.dma_start(out=degn_sb[:], in_=degn_d[:, :])
            z2all = cp.tile([P, npc], F16)
            nstats = len(cfg.groups)
            ssum = cp.tile([P, nstats], F32)
            ssq = cp.tile([P, nstats], F32)

            # balance gather rows across the 4 SWDGE queues (greedy, in
            # issue order: PRE_LO lo-gathers first, then per-group lo/hi)
            qassign = {}
            qload = [0, 0, 0, 0]
            _issue_seq = [(gi, 0) for gi in range(min(4, len(cfg.groups)))]
            for gi in range(len(cfg.groups)):
                if gi + 4 < len(cfg.groups):
                    _issue_seq.append((gi + 4, 0))
                _issue_seq.append((gi, 1))
            for (gi_, h_) in _issue_seq:
                q = int(np.argmin(qload))
                qassign[(gi_, h_)] = q
                qload[q] += int(K2[gi_, h_])

            ngroups = len(cfg.groups)
            group_cpos = []
            cpos = 0
            for gi in range(ngroups):
                group_cpos.append(cpos)
                cpos += int(K2[gi, 0] + K2[gi, 1])

            for l in range(L):
                if l == 0 or no_ag:
                    tab0 = xt_d[0:NC * H0, :]
                    tab1 = xt_d[NC * H0:N, :]
                else:
                    tab0 = ag_out0[l - 1][0:NC * H0, :]
                    tab1 = ag_out1[l - 1][0:NC * H1, :]
                selfbuf = hrm[(l - 1) % 2]
                last = l == L - 1

                # issue order: a window of lo gathers (independent of
                # AG#1) runs on Pool before each hi gather, so the first
                # hi gather's wait on AG#1 is hidden under lo desc-gen.
                # Gathers round-robin the 4 SWDGE queues: each queue's ring
                # feeds its own DMA stream (~8 ns/row each), so 4 queues
                # sustain ~4x the single-queue gather rate.
                gt_tiles = {}

                def issue_lo(gi):
                    cp0 = group_cpos[gi]
                    klo = int(K2[gi, 0])
                    khi = int(K2[gi, 1])
                    gt = gp.tile([P, (klo + khi) * P], F16, name="gt", tag="gt")
                    gt_tiles[gi] = gt
                    if klo and not self_only:
                        nc.gpsimd.dma_gather(
                            gt[:, :klo * P].rearrange("p (c f) -> p c f", f=P),
                            tab0,
                            idx_sb[:, cp0 * 8:(cp0 + klo) * 8],
                            klo * P, klo * P, P, elem_step=P,
                            single_packet=False, queue_num=(2 * gi) % 4)

                def issue_hi(gi):
                    cp0 = group_cpos[gi]
                    klo = int(K2[gi, 0])
                    khi = int(K2[gi, 1])
                    kg = klo + khi
                    gt = gt_tiles[gi]
                    if khi and not self_only:
                        nc.gpsimd.dma_gather(
                            gt[:, klo * P:kg * P].rearrange(
                                "p (c f) -> p c f", f=P),
                            tab1,
                            idx_sb[:, (cp0 + klo) * 8:(cp0 + kg) * 8],
                            khi * P, khi * P, P, elem_step=P,
                            single_packet=False, queue_num=(2 * gi + 1) % 4)

                PRE_LO = 4
                for gi in range(min(PRE_LO, ngroups)):
                    issue_lo(gi)

                for gi, g in enumerate(cfg.groups):
                    gw = sum(cfg.tsize(t) for t in g)
                    goff = g[0] * P
                    cp0 = group_cpos[gi]
                    klo = int(K2[gi, 0])
                    khi = int(K2[gi, 1])
                    kg = klo + khi
                    if gi + PRE_LO < ngroups:
                        issue_lo(gi + PRE_LO)
                    issue_hi(gi)
                    gt = gt_tiles.pop(gi)
                    if l > 0 and kg and not self_only:
                        # deferred from layer l-1: relu(s*x+t) = s*(max(x,-u)+u)
                        # with u=t/s. The clip happens here; the +u moves past
                        # the edge-sum as a rank-1 u x (deg+1) matmul into
                        # psum, and s folds into the psum->zt activation.
                        up_rep = urep[(l - 1) % 2]
                        nc.vector.tensor_tensor(
                            out=gt[:, :kg * P].rearrange(
                                "p (k c) -> p k c", c=P),
                            in0=gt[:, :kg * P].rearrange(
                                "p (k c) -> p k c", c=P),
                            in1=up_rep[:, 0:P].unsqueeze(1).broadcast_to(
                                [P, kg, P]),
                            op=mybir.AluOpType.max)
                    if l > 0:
                        ng_ = len(g)
                        sl_ = selfbuf[:, g[0] * P:(g[0] + ng_) * P]
                        nc.vector.tensor_tensor(
                            out=sl_.rearrange("p (k c) -> p k c", c=P),
                            in0=sl_.rearrange("p (k c) -> p k c", c=P),
                            in1=urep[(l - 1) % 2][:, 0:P].unsqueeze(
                                1).broadcast_to([P, ng_, P]),
                            op=mybir.AluOpType.max)

                    psum = pagg.tile([P, gw], F32, name="psum", tag="psum",
                                     padded_shape=[P, 4 * P], space="PSUM")
                    # one PSUM accumulation group: self matmuls first (start
                    # on the very first), then per-chunk window-segment
                    # matmuls, stop on the last segment.
                    toff = 0
                    for ti, t in enumerate(g):
                        ts_ = cfg.tsize(t)
                        nc.tensor.matmul(
                            out=psum[:, toff:toff + ts_],
                            lhsT=selfbuf[0:ts_, t * P:t * P + P],
                            rhs=ident16[0:ts_, 0:ts_],
                            start=(ti == 0),
                            stop=((self_only or gather_only) and ti == len(g) - 1))
                        toff += ts_
                    if not (self_only or gather_only):
                        if l > 0:
                            # (-u) x (-(deg+1)) = u*(deg+1), the deferred +u
                            nc.tensor.matmul(
                                out=psum[:, 0:gw],
                                lhsT=urow[(l - 1) % 2][0:1, 0:P],
                                rhs=degn_sb[0:1, goff:goff + gw],
                                start=False, stop=False)
                        chunks = sched.group_chunks[gi]
                        nmm = sum(len(c_) for c_ in chunks)
                        s0 = chunks[0][0][1]      # first seg col of group
                        # one batched is_equal builds every one-hot of the
                        # group: oh_all[p, s, c] = (iota[c] == dstl[p, s0+s])
                        oh_all = ohp.tile([P, nmm * P], F16, name="oh", tag="oh")
                        nc.vector.tensor_tensor(
                            out=oh_all[:, :].rearrange("p (s c) -> p s c", c=P),
                            in0=iota16[:, 0:P].unsqueeze(1).broadcast_to(
                                [P, nmm, P]),
                            in1=dstl16[:, s0:s0 + nmm].unsqueeze(2).broadcast_to(
                                [P, nmm, P]),
                            op=mybir.AluOpType.is_equal)
                        mm = 0
                        for j, segs in enumerate(chunks):
                            for (w, scol) in segs:
                                ts_ = cfg.tsize(w)
                                woff = (w - g[0]) * P
                                so = (scol - s0) * P
                                mm += 1
                                nc.tensor.matmul(
                                    out=psum[:, woff:woff + ts_],
                                    lhsT=gt[:, j * P:(j + 1) * P],
                                    rhs=oh_all[:, so:so + ts_],
                                    start=False, stop=(mm == nmm))

                    # ---- MLP ----
                    zt = up.tile([P, gw], F32, name="zt", tag="zt",
                                 padded_shape=[P, 4 * P])
                    zscale = 1.0 if l == 0 else scol_hist[:, l - 1:l]
                    nc.scalar.activation(out=zt[:, :], in_=psum[:, :],
                                         func=copy_op, bias=0.0, scale=zscale)
                    u_t = [up.tile([P, gw], F32, name=f"u{hh}", tag=f"u{hh}",
                                   padded_shape=[P, 4 * P]) for hh in range(2)]
                    for hh in range(2):
                        ps1 = pmlp.tile([P, gw], F32, name="ps1", tag="ps1",
                                        padded_shape=[P, 4 * P], space="PSUM")
                        nc.tensor.matmul(
                            out=ps1[:, :],
                            lhsT=w1_sb[:, l * 2 * P + hh * P:l * 2 * P + hh * P + P],
                            rhs=zt[:, :],
                            start=True, stop=True)
                        nc.scalar.activation(
                            out=u_t[hh][:, :], in_=ps1[:, :], func=relu_op,
                            bias=b1_sb[:, l * 2 + hh:l * 2 + hh + 1], scale=1.0)
                    ps2 = pmlp.tile([P, gw], F32, name="ps2", tag="ps2",
                                    padded_shape=[P, 4 * P], space="PSUM")
                    for hh in range(2):
                        nc.tensor.matmul(
                            out=ps2[:, :],
                            lhsT=w2_sb[:, (l * 2 + hh) * P:(l * 2 + hh) * P + P],
                            rhs=u_t[hh][:, :],
                            start=(hh == 0), stop=(hh == 1))
                    nc.vector.tensor_scalar(
                        out=z2all[:, goff:goff + gw], in0=ps2[:, :],
                        scalar1=b2_sb[:, l:l + 1], scalar2=None,
                        op0=mybir.AluOpType.add)
                    nc.vector.tensor_reduce(
                        out=ssum[:, gi:gi + 1], in_=z2all[:, goff:goff + gw],
                        axis=mybir.AxisListType.X, op=mybir.AluOpType.add)
                    sq_scr = scrp.tile([P, 4 * P], F32, name="sq_scr", tag="sq")
                    nc.scalar.activation(
                        out=sq_scr[:, 0:gw], in_=z2all[:, goff:goff + gw],
                        func=mybir.ActivationFunctionType.Square,
                        bias=zero_sb[:, 0:1],
                        accum_out=ssq[:, gi:gi + 1])

                    if last and not no_segmax:
                        # masked per-(graph x tile) maxes of raw z + BIG (>0),
                        # before BN stats arrive: max commutes with the final
                        # positive-scale affine normalize, applied later to
                        # the 128-col slot-max matrix instead of all nodes.
                        for t in g:
                            ts_ = cfg.tsize(t)
                            S = sched.gm_S[t]
                            o = int(gm_lo_off[t])
                            sb0 = int(gm_slot_base[t])
                            yb = mkp.tile([P, P], F32, name="yb", tag="yb")
                            if ts_ < P:
                                nc.vector.memset(yb[:, ts_:P], 0.0)
                            nc.scalar.activation(
                                out=yb[:, 0:ts_],
                                in_=z2all[:, t * P:t * P + ts_],
                                func=ident_op, bias=big_sb[:, 0:1], scale=1.0)
                            ge = mkp.tile([P, (S + 1) * P], F32, name="ge",
                                          tag="ge",
                                          padded_shape=[P, (SMAX + 1) * P])
                            nc.vector.tensor_tensor(
                                out=ge[:, :(S + 1) * P].rearrange(
                                    "p (s c) -> p s c", c=P),
                                in0=iota_sb[:, 0:P].unsqueeze(1).broadcast_to(
                                    [P, S + 1, P]),
                                in1=gmlo_sb[:, o:o + S + 1].unsqueeze(
                                    2).broadcast_to([P, S + 1, P]),
                                op=mybir.AluOpType.is_ge)
                            inr = mkp.tile([P, S * P], F32, name="inr",
                                           tag="inr",
                                           padded_shape=[P, SMAX * P])
                            nc.vector.tensor_tensor(
                                out=inr[:, :S * P], in0=ge[:, 0:S * P],
                                in1=ge[:, P:(S + 1) * P],
                                op=mybir.AluOpType.subtract)
                            # prod reuses ge's buffer (ge is consumed)
                            nc.vector.tensor_tensor(
                                out=ge[:, :S * P].rearrange(
                                    "p (s c) -> p s c", c=P),
                                in0=yb[:, 0:P].unsqueeze(1).broadcast_to(
                                    [P, S, P]),
                                in1=inr[:, :S * P].rearrange(
                                    "p (s c) -> p s c", c=P),
                                op=mybir.AluOpType.mult)
                            nc.vector.tensor_reduce(
                                out=gmM[:, sb0:sb0 + S],
                                in_=ge[:, :S * P].rearrange(
                                    "p (s c) -> p s c", c=P),
                                axis=mybir.AxisListType.X,
                                op=mybir.AluOpType.max)

                    if not last:
                        # transpose raw z tiles to row-major as each group's
                        # MLP lands; stage-0 AllGather fires mid-layer (after
                        # group 6 covers H0) so next-layer lo gathers overlap
                        # this layer's tail. Normalize is deferred (clip + u
                        # rank-1 + s fold) to the consumers next layer.
                        hout = hrm[l % 2]
                        for t in g:
                            ts_ = cfg.tsize(t)
                            zn = znp.tile([P, P], F16, name="zn16", tag="zn16")
                            nc.scalar.activation(out=zn[:, 0:ts_],
                                                 in_=z2all[:, t * P:t * P + ts_],
                                                 func=copy_op, bias=0.0,
                                                 scale=1.0)
                            tp = ptp.tile([P, P], F16, name="tp", tag="tp",
                                          space="PSUM")
                            nc.tensor.transpose(out=tp[0:ts_, :],
                                                in_=zn[:, 0:ts_],
                                                identity=ident16[:, :])
                            nc.vector.tensor_copy(
                                out=hout[0:ts_, t * P:t * P + P],
                                in_=tp[0:ts_, :])
                        if g[0] <= H0 // P - 1 <= g[-1]:
                            nc.sync.dma_start(
                                out=ag_in0[l][0:H0, :].rearrange(
                                    "(t p) f -> p t f", p=P),
                                in_=hout[:, 0:H0].rearrange(
                                    "p (t f) -> p t f", f=P))
                            if not no_ag:
                                nc.gpsimd.collective_compute(
                                    "AllGather", mybir.AluOpType.bypass,
                                    replica_groups=rg,
                                    ins=[ag_in0[l][:, :]],
                                    outs=[ag_out0[l][:, :]])
                        if gi == len(cfg.groups) - 1:
                            n1full = H1 // P
                            nc.sync.dma_start(
                                out=ag_in1[l][0:n1full * P, :].rearrange(
                                    "(t p) f -> p t f", p=P),
                                in_=hout[:, H0:H0 + n1full * P].rearrange(
                                    "p (t f) -> p t f", f=P))
                            if H1 % P:
                                ts_ = H1 % P
                                nc.sync.dma_start(
                                    out=ag_in1[l][n1full * P:H1, :],
                                    in_=hout[0:ts_,
                                             H0 + n1full * P:H0 + n1full * P + P])

                # ---- BN stats allreduce (issued before AG#1 so the tiny
                # latency-critical AR isn't queued behind the bulk AG on the
                # collective cores; urep gates the next layer's first clip) ----
                ar_sb = sp.tile([P, 2], F32, name="ar_sb", tag="ar")
                nc.vector.tensor_reduce(out=ar_sb[:, 0:1], in_=ssum[:, :],
                                        axis=mybir.AxisListType.X,
                                        op=mybir.AluOpType.add)
                nc.vector.tensor_reduce(out=ar_sb[:, 1:2], in_=ssq[:, :],
                                        axis=mybir.AxisListType.X,
                                        op=mybir.AluOpType.add)
                if no_ar:
                    arr = ar_sb
                else:
                    nc.sync.dma_start(out=ar_in[l][:, :], in_=ar_sb[:, :])
                    nc.gpsimd.collective_compute(
                        "AllReduce", mybir.AluOpType.add, replica_groups=rg,
                        ins=[ar_in[l][:, :]], outs=[ar_out[l][:, :]])
                    arr = sp.tile([P, 2], F32, name="arr", tag="ar")
                    nc.sync.dma_start(out=arr[:, :], in_=ar_out[l][:, :])
                if not last and not no_ag:
                    nc.gpsimd.collective_compute(
                        "AllGather", mybir.AluOpType.bypass, replica_groups=rg,
                        ins=[ag_in1[l][:, :]], outs=[ag_out1[l][:, :]])

                stat = sp.tile([P, 6], F32, name="stat", tag="stat")
                mean, msq, var, istd, s_col, t_col = [stat[:, i:i + 1] for i in range(6)]
                nc.vector.tensor_scalar(out=mean, in0=arr[:, 0:1], scalar1=inv_n,
                                        scalar2=None, op0=mybir.AluOpType.mult)
                nc.vector.tensor_scalar(out=msq, in0=arr[:, 1:2], scalar1=inv_n,
                                        scalar2=None, op0=mybir.AluOpType.mult)
                sq_t = sp.tile([P, 2], F32, name="sq_t", tag="sq_t")
                nc.vector.tensor_tensor(out=sq_t[:, 0:1], in0=mean, in1=mean,
                                        op=mybir.AluOpType.mult)
                nc.vector.tensor_tensor(out=var, in0=msq, in1=sq_t[:, 0:1],
                                        op=mybir.AluOpType.subtract)
                std_t = sp.tile([P, 2], F32, name="std_t", tag="sq_t")
                nc.scalar.activation(out=std_t[:, 0:1], in_=var,
                                     func=mybir.ActivationFunctionType.Sqrt,
                                     bias=eps_sb[:, 0:1], scale=1.0)
                nc.vector.reciprocal(out=istd, in_=std_t[:, 0:1])
                nc.vector.tensor_tensor(out=s_col, in0=gam_sb[:, l:l + 1], in1=istd,
                                        op=mybir.AluOpType.mult)
                nc.vector.tensor_tensor(out=sq_t[:, 1:2], in0=mean, in1=s_col,
                                        op=mybir.AluOpType.mult)
                nc.vector.tensor_tensor(out=t_col, in0=bet_sb[:, l:l + 1],
                                        in1=sq_t[:, 1:2],
                                        op=mybir.AluOpType.subtract)
                if not last:
                    # deferred-normalize params for layer l+1: u = t/s as a
                    # row broadcast to all partitions, s per-feature column
                    nc.vector.tensor_copy(out=scol_hist[:, l:l + 1], in_=s_col)
                    uc = sp.tile([P, 2], F32, name="uc", tag="uc")
                    nc.vector.reciprocal(out=uc[:, 0:1], in_=s_col)
                    nc.vector.tensor_tensor(out=uc[:, 1:2], in0=t_col,
                                            in1=uc[:, 0:1],
                                            op=mybir.AluOpType.mult)
                    u16 = sp.tile([P, 1], F16, name="u16", tag="u16")
                    nc.vector.tensor_scalar(out=u16[:, 0:1], in0=uc[:, 1:2],
                                            scalar1=-1.0, scalar2=None,
                                            op0=mybir.AluOpType.mult)
                    urtp = ptp.tile([P, P], F16, name="urtp", tag="tp",
                                    space="PSUM")
                    nc.tensor.transpose(out=urtp[0:1, :], in_=u16[:, 0:1],
                                        identity=ident16[:, :])
                    nc.vector.tensor_copy(out=urow[l % 2][0:1, :],
                                          in_=urtp[0:1, :])
                    # broadcast -u to all partitions via PE (ones x u_row);
                    # partition_broadcast would stall the in-order Pool queue
                    # on the AR result and delay next-layer gather desc-gen
                    ub_ps = pagg.tile([P, P], F32, name="ubps", tag="psum",
                                      padded_shape=[P, 4 * P], space="PSUM")
                    nc.tensor.matmul(out=ub_ps[:, :],
                                     lhsT=ones_row[0:1, 0:P],
                                     rhs=urow[l % 2][0:1, :],
                                     start=True, stop=True)
                    nc.vector.tensor_copy(out=urep[l % 2][:, :],
                                          in_=ub_ps[:, :])

                if last and no_segmax:
                    gz = cp.tile([G, P], F32, name="gz")
                    nc.vector.memset(gz[:, :], 0.0)
                    nc.sync.dma_start(out=gmax_out[:, :], in_=gz[:, :])
                else:
                    # ---- finish global max pool: normalize the slot-max
                    # matrix (max commutes with the positive-scale affine),
                    # select each graph's k-th slot via one-hot matmuls,
                    # max-combine, AllReduce-max across cores.
                    stmp = cp.tile([P, 2], F32, name="gmt")
                    nc.vector.tensor_scalar(out=stmp[:, 0:1], in0=s_col,
                                            scalar1=GM_BIG, scalar2=None,
                                            op0=mybir.AluOpType.mult)
                    nc.vector.tensor_tensor(out=stmp[:, 1:2], in0=t_col,
                                            in1=stmp[:, 0:1],
                                            op=mybir.AluOpType.subtract)
                    gmMn = cp.tile([P, P], F32, name="gmMn")
                    nc.scalar.activation(out=gmMn[:, :], in_=gmM[:, :],
                                         func=ident_op, bias=stmp[:, 1:2],
                                         scale=s_col)
                    nc.vector.memset(gmMn[:, 127:128], GM_SENT)
                    tpg = pagg.tile([P, P], F32, name="gmtp", tag="psum",
                                    padded_shape=[P, 4 * P], space="PSUM")
                    nc.tensor.transpose(out=tpg[:, :], in_=gmMn[:, :],
                                        identity=ident_sb[:, :])
                    gmMT = cp.tile([P, P], F32, name="gmMT")
                    nc.vector.tensor_copy(out=gmMT[:, :], in_=tpg[:, :])
                    gcur = [cp.tile([G, P], F32, name=f"gc{i}")
                            for i in range(2)]
                    for k in range(GMK):
                        psg = pmlp.tile([G, P], F32, name="gmps", tag="ps1",
                                        padded_shape=[P, 4 * P],
                                        space="PSUM")
                        nc.tensor.matmul(out=psg[:, :],
                                         lhsT=gmsel_sb[:, k * G:(k + 1) * G],
                                         rhs=gmMT[:, :],
                                         start=True, stop=True)
                        if k == 0:
                            nc.vector.tensor_copy(out=gcur[0][:, :],
                                                  in_=psg[:, :])
                        else:
                            nc.vector.tensor_tensor(
                                out=gcur[k % 2][:, :],
                                in0=gcur[(k + 1) % 2][:, :], in1=psg[:, :],
                                op=mybir.AluOpType.max)
                    gfin = gcur[(GMK - 1) % 2]
                    if no_ar:
                        nc.sync.dma_start(out=gmax_out[:, :], in_=gfin[:, :])
                    else:
                        nc.sync.dma_start(out=gm_in[:, :], in_=gfin[:, :])
                        nc.gpsimd.collective_compute(
                            "AllReduce", mybir.AluOpType.max,
                            replica_groups=rg,
                            ins=[gm_in[:, :]], outs=[gm_out[:, :]])
                        gmax2 = cp.tile([G, P], F32, name="gmax2")
                        nc.sync.dma_start(out=gmax2[:, :], in_=gm_out[:, :])
                        nc.sync.dma_start(out=gmax_out[:, :], in_=gmax2[:, :])

    nc.compile()
    return nc


def prep_inputs(cfg: Cfg, sched: Sched, x, W1, b1, W2, b2, gamma, beta):
    """Per-core input maps. Host does data layout only: transpose/cast/shard."""
    N, L, ntiles, npc = cfg.N, cfg.L, cfg.ntiles, cfg.npc
    x = np.asarray(x, np.float32)
    x16 = np.ascontiguousarray(x.astype(np.float16))
    H0 = 20 * P
    xr = np.ascontiguousarray(np.concatenate(
        [x16[c * npc:c * npc + H0] for c in range(NC)] +
        [x16[c * npc + H0:(c + 1) * npc] for c in range(NC)], axis=0))
    iota = np.broadcast_to(np.arange(P, dtype=np.float32), (P, P)).copy()
    ident = np.eye(P, dtype=np.float32)
    w1 = np.ascontiguousarray(np.transpose(np.asarray(W1, np.float32), (1, 0, 2))
                              ).reshape(P, L * 2 * P)
    w2 = np.ascontiguousarray(np.transpose(
        np.asarray(W2, np.float32).reshape(L, 2, P, P), (2, 0, 1, 3))
        ).reshape(P, L * 2 * P)
    b1r = np.ascontiguousarray(np.transpose(
        np.asarray(b1, np.float32).reshape(L, 2, P), (2, 0, 1))).reshape(P, L * 2)
    b2r = np.ascontiguousarray(np.asarray(b2, np.float32).T)
    gam = np.ascontiguousarray(np.asarray(gamma, np.float32).T)
    bet = np.ascontiguousarray(np.asarray(beta, np.float32).T)

    in_maps = []
    for c in range(NC):
        shard = x16[c * npc:(c + 1) * npc]
        xh = np.zeros((P, ntiles * P), np.float16)
        nf = npc // P
        xh[:, :nf * P] = shard[:nf * P].reshape(nf, P, P).transpose(1, 0, 2).reshape(P, nf * P)
        if npc % P:
            xh[0:npc % P, nf * P:(nf + 1) * P] = shard[nf * P:]
        in_maps.append({
            "x_hrm": xh, "x_tab": xr,
            "idx16": sched.idx16[c], "gmlo": sched.gm_lo[c],
            "gmsel": sched.gm_sel[c], "degn": sched.degn[c],
            "dstl": sched.dstl[c],
            "iota": iota, "ident": ident,
            "w1": w1, "w2": w2, "b1": b1r, "b2": b2r, "gam": gam, "bet": bet,
        })
    return in_maps


# ---------------------------------------------------------------------------
# PJRT runner: compile once, stage inputs on device, reuse the executable
# ---------------------------------------------------------------------------

def make_runner(nc, n_cores=NC):
    """Build a reusable jitted executable for the Bass module (axon/PJRT)."""
    import jax
    import numpy as _np
    from jax.sharding import Mesh, PartitionSpec
    from jax.experimental.shard_map import shard_map
    import concourse.bass2jax as b2j

    b2j.install_neuronx_cc_hook()
    partition_name = nc.partition_id_tensor.name if nc.partition_id_tensor else None
    in_names, out_names, out_avals, zero_shapes = [], [], [], []
    for alloc in nc.m.functions[0].allocations:
        if not isinstance(alloc, mybir.MemoryLocationSet):
            continue
        name = alloc.memorylocations[0].name
        if alloc.kind == "ExternalInput":
            if name != partition_name:
                in_names.append(name)
        elif alloc.kind == "ExternalOutput":
            out_names.append(name)
            shape = tuple(alloc.tensor_shape)
            dtype = mybir.dt.np(alloc.dtype)
            out_avals.append(jax.core.ShapedArray(shape, dtype))
            zero_shapes.append((shape, dtype))
    n_params = len(in_names)
    all_in = list(in_names) + list(out_names)
    if partition_name is not None:
        all_in.append(partition_name)

    def _body(*args):
        operands = list(args)
        if partition_name is not None:
            operands.append(b2j.partition_id_tensor())
        outs = b2j._bass_exec_p.bind(
            *operands,
            out_avals=tuple(out_avals),
            in_names=tuple(all_in),
            out_names=tuple(out_names),
            lowering_input_output_aliases=(),
            sim_require_finite=True,
            sim_require_nnan=True,
            nc=nc,
        )
        return tuple(outs)

    devices = jax.devices()[:n_cores]
    mesh = Mesh(_np.asarray(devices), ("core",))
    donate = tuple(range(n_params, n_params + len(out_names)))
    in_specs = (PartitionSpec("core"),) * (n_params + len(out_names))
    out_specs = (PartitionSpec("core"),) * len(out_names)
    sharded = jax.jit(
        shard_map(_body, mesh=mesh, in_specs=in_specs, out_specs=out_specs,
                  check_rep=False),
        donate_argnums=donate, keep_unused=True)
    return sharded, in_names, out_names, zero_shapes, mesh


_CACHE = {}


def _get_compiled(cfg, edge_index, batch):
    key = (cfg.N, cfg.E, cfg.L, cfg.G,
           hashlib.blake2b(np.ascontiguousarray(edge_index).tobytes(),
                           digest_size=16).hexdigest(),
           hashlib.blake2b(np.ascontiguousarray(batch).tobytes(),
                           digest_size=16).hexdigest())
    if key not in _CACHE:
        sched = build_schedule(cfg, edge_index, batch)
        nc = build_nc(cfg, sched)
        runner = make_runner(nc, NC)
        _CACHE[key] = (sched, nc, runner)
    return _CACHE[key]


def kernel(x, edge_index, batch, num_graphs, W1, b1, W2, b2, gamma, beta):
    """GIN forward on 8 TRN2 NeuronCores. Full inputs in, full output out."""
    import jax
    from jax.sharding import NamedSharding, PartitionSpec

    x = np.asarray(x, np.float32)
    edge_index = np.asarray(edge_index)
    batch = np.asarray(batch)
    G = int(np.asarray(num_graphs))
    cfg = Cfg(N=x.shape[0], E=edge_index.shape[1], L=np.asarray(W1).shape[0], G=G)

    sched, nc, (sharded, in_names, out_names, zero_shapes, mesh) = \
        _get_compiled(cfg, edge_index, batch)

    in_maps = prep_inputs(cfg, sched, x, W1, b1, W2, b2, gamma, beta)
    sh = NamedSharding(mesh, PartitionSpec("core"))
    concat_in = [np.concatenate([np.asarray(in_maps[c][n]) for c in range(NC)],
                                axis=0) for n in in_names]
    dev_in = [jax.device_put(a, sh) for a in concat_in]
    zeros = [jax.device_put(np.zeros((NC * s[0], *s[1:]), d), sh)
             for s, d in zero_shapes]
    outs = sharded(*dev_in, *zeros)
    gmaxT = np.asarray(outs[out_names.index("gmaxT")])  # [NC*G, P]
    out = np.ascontiguousarray(gmaxT[:G].astype(np.float32))
    # match jax segment_max: empty segments are -inf (sentinel -30000)
    out[out <= -20000.0] = -np.inf
    return out



# revision 43
# speedup vs baseline: 1.1000x; 1.1000x over previous
"""GIN (MoMuGNN) message-passing kernel for 8 TRN2 NeuronCores.

Full inputs in, full output out. All graph compute runs on device:
per-layer edge gather (SWDGE), one-hot scatter-add matmuls into PSUM,
MLP, batch-norm (stats via AllReduce), inter-layer fp16 AllGather of
node features, and the final per-graph segment-max (transpose-gather +
max reduces + AllReduce-max). Host work is limited to data layout
(transpose/cast/shard) and edge-schedule construction, cached per graph.

Edge gathers are descriptor-rate-bound (~8 ns per gathered row on the
GPSIMD SWDGE path), so the schedule minimizes gathered rows: edges are
chunked per (dst-core, 4-window group, src-half) bucket — group-level
chunking needs ~7% padding vs ~16% for per-window chunking — and each
128-edge chunk scatters via one narrow [128x128] one-hot matmul per
128-dst window segment it touches.
"""

import hashlib
import numpy as np
from dataclasses import dataclass, field

import concourse.tile as tile
from concourse import bacc, mybir

P = 128
NC = 8
BN_EPS = 1e-5
F32 = mybir.dt.float32
F16 = mybir.dt.float16


@dataclass
class Cfg:
    N: int
    E: int
    L: int
    G: int
    F: int = 128

    @property
    def npc(self):
        return self.N // NC

    @property
    def half(self):
        return self.N // 2

    @property
    def ntiles(self):
        return (self.npc + P - 1) // P

    def tsize(self, t):
        return min(P, self.npc - t * P)

    @property
    def groups(self):
        gs = []
        t = 0
        while t < self.ntiles:
            gs.append(list(range(t, min(t + 4, self.ntiles))))
            t += 4
        return gs


def _wrap_idx16(flat_idx: np.ndarray, nchunks: int) -> np.ndarray:
    """[nchunks*128] uint16 -> [128, nchunks*8] int16 in the wrapped layout
    dma_gather expects (16-partition wrap, replicated to 128)."""
    w16 = np.zeros((16, nchunks * 8), np.uint16)
    fi = flat_idx.reshape(nchunks * 8, 16)
    w16[:, :] = fi.T
    return np.tile(w16, (8, 1)).view(np.int16)


@dataclass
class Sched:
    K2: np.ndarray         # [ngroups, 2] chunks per (group, src-half), max over cores
    group_chunks: list     # per group: list (consumption order) of seg lists [(w, segcol)]
    nseg: int
    total_chunks: int
    idx16: list            # per core: [128, total_chunks*8] int16 wrapped
    dstl: list             # per core: [128, nseg] fp32 (window-local dst, -1 pad)
    # ---- segment-max mask/select schedule ----
    # per (core, tile): slots = (graph x tile) spans, padded to the per-tile
    # max count S_t over cores. gm_lo holds each tile's S_t+1 ascending span
    # boundaries; gm_sel holds K one-hot slot->graph selection matrices.
    gm_S: list = field(default_factory=list)        # per tile: padded slot count
    gm_lo: list = field(default_factory=list)       # per core: [128, nlocol] f32
    gm_sel: list = field(default_factory=list)      # per core: [128, K*G] f32
    gm_K: int = 0
    gm_nslots: int = 0
    gm_nlocol: int = 0
    degn: list = field(default_factory=list)        # per core: [1, npc] f16, -(deg+1)


def build_schedule(cfg: Cfg, edge_index: np.ndarray, batch: np.ndarray) -> Sched:
    """Bucket edges per (dst-core, 4-window group, src-half); within a bucket
    edges are sorted by (window, src). Chunks are 128 edges; a chunk that
    straddles windows gets one one-hot segment per window (union over cores
    so the SPMD instruction stream is uniform; absent windows get all -1
    dst columns). Also builds the segment-max gather schedule."""
    src = edge_index[0].astype(np.int64)
    dst = edge_index[1].astype(np.int64)
    npc, half, ntiles = cfg.npc, cfg.half, cfg.ntiles
    groups = cfg.groups
    ngroups = len(groups)
    core = dst // npc
    loc = dst % npc
    wi = loc // P                  # window (=tile) within core
    dl = loc - wi * P              # dst local within window
    gi_of_w = np.zeros(ntiles, np.int64)
    for i, g in enumerate(groups):
        for w in g:
            gi_of_w[w] = i
    gidx = gi_of_w[wi]
    # src-half split at tile-aligned local row H0 so AllGather can be issued
    # in two stages and lo gathers start after stage 0
    H0 = 28 * P                     # 3584 local rows -> table0 (stage-0 AG
                                    # issues after group 6, mid-layer)
    sc_ = src // npc
    sr_ = src % npc
    hf = (sr_ >= H0).astype(np.int64)

    order = np.lexsort((src, wi, hf, gidx, core))
    cs = core[order]
    gs_ = gidx[order]
    hs = hf[order]
    ws_ = wi[order]
    tab_row = np.where(hf == 0, sc_ * H0 + sr_,
                       sc_ * (npc - H0) + (sr_ - H0))
    srcs = tab_row[order]
    dls = dl[order]
    key = (cs * ngroups + gs_) * 2 + hs
    bounds = np.searchsorted(key, np.arange(NC * ngroups * 2 + 1))
    buckets = {}
    cnt = np.zeros((NC, ngroups, 2), np.int64)
    for c in range(NC):
        for gi in range(ngroups):
            for h in range(2):
                k = (c * ngroups + gi) * 2 + h
                a, b = bounds[k], bounds[k + 1]
                buckets[(c, gi, h)] = (srcs[a:b], dls[a:b], ws_[a:b])
                cnt[c, gi, h] = b - a

    K2 = np.zeros((ngroups, 2), np.int64)
    for gi in range(ngroups):
        for h in range(2):
            m = cnt[:, gi, h].max()
            K2[gi, h] = (m + P - 1) // P if m > 0 else 0
        if K2[gi].sum() == 0:
            K2[gi, 0] = 1

    # segments: per (group, half, chunk) the union of windows over cores
    group_chunks = [[] for _ in range(ngroups)]
    seg_cols = 0
    chunk_cols = 0
    for gi in range(ngroups):
        for h in range(2):
            for j in range(int(K2[gi, h])):
                wins = set()
                for c in range(NC):
                    _s, _d, wv = buckets[(c, gi, h)]
                    seg = wv[j * P:(j + 1) * P]
                    if len(seg):
                        wins.update(np.unique(seg).tolist())
                if not wins:
                    wins = {groups[gi][0]}
                segs = [(int(w), seg_cols + k) for k, w in enumerate(sorted(wins))]
                seg_cols += len(segs)
                group_chunks[gi].append(segs)
                chunk_cols += 1
    nseg = seg_cols
    total_chunks = chunk_cols

    idx16, dstl = [], []
    for c in range(NC):
        flat_idx = np.zeros(total_chunks * P, np.uint16)
        flat_dl = np.full((P, nseg), -1.0, np.float32)
        pos = 0
        for gi in range(ngroups):
            ci = 0
            for h in range(2):
                s_arr, d_arr, w_arr = buckets[(c, gi, h)]
                n = len(s_arr)
                for j in range(int(K2[gi, h])):
                    rows = slice(j * P, (j + 1) * P)
                    sseg = s_arr[rows]
                    flat_idx[pos * P:pos * P + len(sseg)] = sseg.astype(np.uint16)
                    dseg = d_arr[rows]
                    wseg = w_arr[rows]
                    for (w, scol) in group_chunks[gi][ci]:
                        col = np.full(P, -1.0, np.float32)
                        m = wseg == w
                        col[:len(dseg)][m] = dseg[m].astype(np.float32)
                        flat_dl[:, scol] = col
                    pos += 1
                    ci += 1
        assert pos == total_chunks
        idx16.append(_wrap_idx16(flat_idx, total_chunks))
        dstl.append(flat_dl)

    # ---- segment-max mask/select schedule ---------------------------------
    # Graph spans are contiguous in local node order (batch sorted), so each
    # (graph, tile) incidence is a column span [lo, hi) inside the tile; the
    # spans of a tile's slots partition its columns. The per-tile slot count
    # is padded to S_t = max over cores (zero-width spans); per-core span
    # boundaries flow in as data (gm_lo). Per-graph combine: each graph's
    # slots are consecutive; K one-hot matrices select the k-th slot of each
    # graph (sentinel column 127 for absent/padded picks), max-combined.
    batch = np.asarray(batch, np.int64)
    G = cfg.G
    starts = np.searchsorted(batch, np.arange(G))
    ends = np.searchsorted(batch, np.arange(1, G + 1))
    core_tile_slots = []
    for c in range(NC):
        c0, c1 = c * npc, (c + 1) * npc
        per_tile = [[] for _ in range(ntiles)]
        for g in range(G):
            a, b = max(int(starts[g]), c0), min(int(ends[g]), c1)
            if b <= a:
                continue
            la, lb = a - c0, b - c0
            for t in range(la // P, (lb - 1) // P + 1):
                lo = max(la - t * P, 0)
                hi = min(lb - t * P, cfg.tsize(t))
                per_tile[t].append((g, lo, hi))
        core_tile_slots.append(per_tile)
    gm_S = [max(len(core_tile_slots[c][t]) for c in range(NC))
            for t in range(ntiles)]
    slot_base = np.concatenate([[0], np.cumsum(gm_S)]).astype(int)
    gm_nslots = int(slot_base[-1])
    assert gm_nslots <= 127, gm_nslots
    SENT = 127
    gm_nlocol = int(sum(s + 1 for s in gm_S))
    gm_lo, gslots_all = [], []
    gm_K = 1
    for c in range(NC):
        lo_flat = np.zeros(gm_nlocol, np.float32)
        gslots = {}
        off = 0
        for t in range(ntiles):
            ts_ = float(cfg.tsize(t))
            slots = core_tile_slots[c][t]
            col = np.full(gm_S[t] + 1, ts_, np.float32)
            for i, (g, lo, hi) in enumerate(slots):
                col[i] = float(lo)
                gslots.setdefault(g, []).append(int(slot_base[t]) + i)
            lo_flat[off:off + gm_S[t] + 1] = col
            off += gm_S[t] + 1
        gslots_all.append(gslots)
        if gslots:
            gm_K = max(gm_K, max(len(v) for v in gslots.values()))
        gm_lo.append(np.broadcast_to(lo_flat, (P, gm_nlocol)).copy())
    gm_sel = []
    for c in range(NC):
        sel = np.zeros((P, gm_K * G), np.float32)
        gslots = gslots_all[c]
        for g in range(G):
            sl = gslots.get(g, [])
            for k in range(gm_K):
                s = sl[k] if k < len(sl) else SENT
                sel[s, k * G + g] = 1.0
        gm_sel.append(sel)

    degn = []
    for c in range(NC):
        cnt_c = np.bincount(loc[core == c], minlength=npc).astype(np.float64)
        degn.append((-(cnt_c + 1.0)).astype(np.float16).reshape(1, npc))

    return Sched(K2=K2, group_chunks=group_chunks, nseg=nseg,
                 total_chunks=total_chunks, idx16=idx16, dstl=dstl,
                 gm_S=gm_S, gm_lo=gm_lo, gm_sel=gm_sel, gm_K=gm_K,
                 gm_nslots=gm_nslots, gm_nlocol=gm_nlocol, degn=degn)


def build_nc(cfg: Cfg, sched: Sched, *, no_ar=False, no_ag=False,
             self_only=False, no_segmax=False, gather_only=False):
    npc, ntiles, L, N, G = cfg.npc, cfg.ntiles, cfg.L, cfg.N, cfg.G
    half = cfg.half
    TC = sched.total_chunks
    NSEG = sched.nseg
    K2 = sched.K2
    GMK = sched.gm_K
    SMAX = max(sched.gm_S)
    gm_lo_off = np.concatenate([[0], np.cumsum([s + 1 for s in sched.gm_S])])
    gm_slot_base = np.concatenate([[0], np.cumsum(sched.gm_S)]).astype(int)
    GM_BIG = 4096.0       # |raw z| < ~40, so z+BIG stays positive; fp32 step
    GM_SENT = -30000.0    # at 4096 is 2.4e-4 -- negligible vs the 2e-2 gate
    relu_op = mybir.ActivationFunctionType.Relu
    copy_op = mybir.ActivationFunctionType.Copy
    ident_op = mybir.ActivationFunctionType.Identity

    nc = bacc.Bacc("TRN2", target_bir_lowering=False, debug=False, num_devices=NC,
                   num_swdge_queues=4)

    xh_d = nc.dram_tensor("x_hrm", [P, ntiles * P], F16, kind="ExternalInput")
    xt_d = nc.dram_tensor("x_tab", [N, P], F16, kind="ExternalInput")
    idx_d = nc.dram_tensor("idx16", [P, TC * 8], mybir.dt.int16, kind="ExternalInput")
    gmlo_d = nc.dram_tensor("gmlo", [P, sched.gm_nlocol], F32,
                            kind="ExternalInput")
    gmsel_d = nc.dram_tensor("gmsel", [P, GMK * G], F32, kind="ExternalInput")
    degn_d = nc.dram_tensor("degn", [1, npc], F16, kind="ExternalInput")
    dstl_d = nc.dram_tensor("dstl", [P, NSEG], F32, kind="ExternalInput")
    iota_d = nc.dram_tensor("iota", [P, P], F32, kind="ExternalInput")
    ident_d = nc.dram_tensor("ident", [P, P], F32, kind="ExternalInput")
    w1_d = nc.dram_tensor("w1", [P, L * 2 * P], F32, kind="ExternalInput")
    w2_d = nc.dram_tensor("w2", [P, L * 2 * P], F32, kind="ExternalInput")
    b1_d = nc.dram_tensor("b1", [P, L * 2], F32, kind="ExternalInput")
    b2_d = nc.dram_tensor("b2", [P, L], F32, kind="ExternalInput")
    gam_d = nc.dram_tensor("gam", [P, L], F32, kind="ExternalInput")
    bet_d = nc.dram_tensor("bet", [P, L], F32, kind="ExternalInput")

    gmax_out = nc.dram_tensor("gmaxT", [G, P], F32, kind="ExternalOutput")

    H0 = 28 * P
    H1 = npc - H0
    ag_in0 = [nc.dram_tensor(f"ag_in0_{l}", [H0, P], F16, kind="Internal")
              for l in range(L - 1)]
    ag_in1 = [nc.dram_tensor(f"ag_in1_{l}", [H1, P], F16, kind="Internal")
              for l in range(L - 1)]
    ag_out0 = [nc.dram_tensor(f"ag_out0_{l}", [NC * H0, P], F16,
                              kind="Internal", addr_space="Shared")
               for l in range(L - 1)]
    ag_out1 = [nc.dram_tensor(f"ag_out1_{l}", [NC * H1, P], F16,
                              kind="Internal", addr_space="Shared")
               for l in range(L - 1)]
    ar_in = [nc.dram_tensor(f"ar_in_{l}", [P, 2], F32, kind="Internal")
             for l in range(L)]
    ar_out = [nc.dram_tensor(f"ar_out_{l}", [P, 2], F32, kind="Internal",
                             addr_space="Shared") for l in range(L)]
    gm_in = nc.dram_tensor("gm_in", [G, P], F32, kind="Internal")
    gm_out = nc.dram_tensor("gm_out", [G, P], F32, kind="Internal",
                            addr_space="Shared")
    rg = [list(range(NC))]

    inv_n = 1.0 / N

    with tile.TileContext(nc) as tc:
        with tc.tile_pool(name="const", bufs=1) as cp, \
             tc.tile_pool(name="gath", bufs=5) as gp, \
             tc.tile_pool(name="oh", bufs=2) as ohp, \
             tc.tile_pool(name="zn", bufs=3) as znp, \
             tc.tile_pool(name="u", bufs=2) as up, \
             tc.tile_pool(name="small", bufs=8) as sp, \
             tc.tile_pool(name="scr", bufs=2) as scrp, \
             tc.tile_pool(name="mk", bufs=2) as mkp, \
             tc.tile_pool(name="ps_agg", bufs=2, space="PSUM") as pagg, \
             tc.tile_pool(name="ps_mlp", bufs=2, space="PSUM") as pmlp, \
             tc.tile_pool(name="ps_tp", bufs=2, space="PSUM") as ptp:

            # ---- persistent SBUF ----
            idx_sb = cp.tile([P, TC * 8], mybir.dt.int16)
            nc.sync.dma_start(out=idx_sb[:], in_=idx_d[:, :])
            gmlo_sb = cp.tile([P, sched.gm_nlocol], F32)
            nc.sync.dma_start(out=gmlo_sb[:], in_=gmlo_d[:, :])
            gmsel_sb = cp.tile([P, GMK * G], F32)
            nc.sync.dma_start(out=gmsel_sb[:], in_=gmsel_d[:, :])
            dstl_sb = cp.tile([P, NSEG], F32)
            nc.sync.dma_start(out=dstl_sb[:], in_=dstl_d[:, :])
            iota_sb = cp.tile([P, P], F32)
            nc.sync.dma_start(out=iota_sb[:], in_=iota_d[:, :])
            ident_sb = cp.tile([P, P], F32)
            nc.sync.dma_start(out=ident_sb[:], in_=ident_d[:, :])
            w1_sb = cp.tile([P, L * 2 * P], F32)
            nc.sync.dma_start(out=w1_sb[:], in_=w1_d[:, :])
            w2_sb = cp.tile([P, L * 2 * P], F32)
            nc.sync.dma_start(out=w2_sb[:], in_=w2_d[:, :])
            b1_sb = cp.tile([P, L * 2], F32)
            nc.sync.dma_start(out=b1_sb[:], in_=b1_d[:, :])
            b2_sb = cp.tile([P, L], F32)
            nc.sync.dma_start(out=b2_sb[:], in_=b2_d[:, :])
            gam_sb = cp.tile([P, L], F32)
            nc.sync.dma_start(out=gam_sb[:], in_=gam_d[:, :])
            bet_sb = cp.tile([P, L], F32)
            nc.sync.dma_start(out=bet_sb[:], in_=bet_d[:, :])

            eps_sb = cp.tile([P, 1], F32)
            nc.vector.memset(eps_sb[:], BN_EPS)
            zero_sb = cp.tile([P, 1], F32)
            nc.vector.memset(zero_sb[:], 0.0)
            big_sb = cp.tile([P, 1], F32)
            nc.vector.memset(big_sb[:], GM_BIG)
            gmM = cp.tile([P, P], F32)
            nc.vector.memset(gmM[:], 0.0)
            iota16 = cp.tile([P, P], F16)
            nc.vector.tensor_copy(out=iota16[:], in_=iota_sb[:])
            ident16 = cp.tile([P, P], F16)
            nc.vector.tensor_copy(out=ident16[:], in_=ident_sb[:])
            dstl16 = cp.tile([P, NSEG], F16)
            nc.vector.tensor_copy(out=dstl16[:], in_=dstl_sb[:])
            hrm = [cp.tile([P, ntiles * P], F16, name=f"hrm{i}") for i in range(2)]
            nc.sync.dma_start(out=hrm[1][:], in_=xh_d[:, :])
            nc.vector.memset(hrm[0][:], 0.0)
            scol_hist = cp.tile([P, L], F32)
            urep = [cp.tile([P, P], F16, name=f"urep{i}") for i in range(2)]
            urow = [cp.tile([P, P], F16, name=f"urow{i}") for i in range(2)]
            ones_row = cp.tile([1, P], F16, name="ones_row")
            nc.vector.memset(ones_row[:, :], 1.0)
            degn_sb = cp.tile([1, npc], F16, name="degn")
            nc.sync.dma_start(out=degn_sb[:], in_=degn_d[:, :])
            z2all = cp.tile([P, npc], F16)
            nstats = len(cfg.groups)
            ssum = cp.tile([P, nstats], F32)
            ssq = cp.tile([P, nstats], F32)

            # balance gather rows across the 4 SWDGE queues (greedy, in
            # issue order: PRE_LO lo-gathers first, then per-group lo/hi)
            qassign = {}
            qload = [0, 0, 0, 0]
            _issue_seq = [(gi, 0) for gi in range(min(4, len(cfg.groups)))]
            for gi in range(len(cfg.groups)):
                if gi + 4 < len(cfg.groups):
                    _issue_seq.append((gi + 4, 0))
                _issue_seq.append((gi, 1))
            for (gi_, h_) in _issue_seq:
                q = int(np.argmin(qload))
                qassign[(gi_, h_)] = q
                qload[q] += int(K2[gi_, h_])

            ngroups = len(cfg.groups)
            group_cpos = []
            cpos = 0
            for gi in range(ngroups):
                group_cpos.append(cpos)
                cpos += int(K2[gi, 0] + K2[gi, 1])

            for l in range(L):
                if l == 0 or no_ag:
                    tab0 = xt_d[0:NC * H0, :]
                    tab1 = xt_d[NC * H0:N, :]
                else:
                    tab0 = ag_out0[l - 1][0:NC * H0, :]
                    tab1 = ag_out1[l - 1][0:NC * H1, :]
                selfbuf = hrm[(l - 1) % 2]
                last = l == L - 1

                # issue order: a window of lo gathers (independent of
                # AG#1) runs on Pool before each hi gather, so the first
                # hi gather's wait on AG#1 is hidden under lo desc-gen.
                # Gathers round-robin the 4 SWDGE queues: each queue's ring
                # feeds its own DMA stream (~8 ns/row each), so 4 queues
                # sustain ~4x the single-queue gather rate.
                gt_tiles = {}

                def issue_lo(gi):
                    cp0 = group_cpos[gi]
                    klo = int(K2[gi, 0])
                    khi = int(K2[gi, 1])
                    gt = gp.tile([P, (klo + khi) * P], F16, name="gt", tag="gt")
                    gt_tiles[gi] = gt
                    if klo and not self_only:
                        nc.gpsimd.dma_gather(
                            gt[:, :klo * P].rearrange("p (c f) -> p c f", f=P),
                            tab0,
                            idx_sb[:, cp0 * 8:(cp0 + klo) * 8],
                            klo * P, klo * P, P, elem_step=P,
                            single_packet=False, queue_num=(2 * gi) % 4)

                def issue_hi(gi):
                    cp0 = group_cpos[gi]
                    klo = int(K2[gi, 0])
                    khi = int(K2[gi, 1])
                    kg = klo + khi
                    gt = gt_tiles[gi]
                    if khi and not self_only:
                        nc.gpsimd.dma_gather(
                            gt[:, klo * P:kg * P].rearrange(
                                "p (c f) -> p c f", f=P),
                            tab1,
                            idx_sb[:, (cp0 + klo) * 8:(cp0 + kg) * 8],
                            khi * P, khi * P, P, elem_step=P,
                            single_packet=False, queue_num=(2 * gi + 1) % 4)

                PRE_LO = 4
                for gi in range(min(PRE_LO, ngroups)):
                    issue_lo(gi)

                for gi, g in enumerate(cfg.groups):
                    gw = sum(cfg.tsize(t) for t in g)
                    goff = g[0] * P
                    cp0 = group_cpos[gi]
                    klo = int(K2[gi, 0])
                    khi = int(K2[gi, 1])
                    kg = klo + khi
                    if gi + PRE_LO < ngroups:
                        issue_lo(gi + PRE_LO)
                    issue_hi(gi)
                    gt = gt_tiles.pop(gi)
                    if l > 0 and kg and not self_only:
                        # deferred from layer l-1: relu(s*x+t) = s*(max(x,-u)+u)
                        # with u=t/s. The clip happens here; the +u moves past
                        # the edge-sum as a rank-1 u x (deg+1) matmul into
                        # psum, and s folds into the psum->zt activation.
                        up_rep = urep[(l - 1) % 2]
                        nc.vector.tensor_tensor(
                            out=gt[:, :kg * P].rearrange(
                                "p (k c) -> p k c", c=P),
                            in0=gt[:, :kg * P].rearrange(
                                "p (k c) -> p k c", c=P),
                            in1=up_rep[:, 0:P].unsqueeze(1).broadcast_to(
                                [P, kg, P]),
                            op=mybir.AluOpType.max)
                    if l > 0:
                        ng_ = len(g)
                        sl_ = selfbuf[:, g[0] * P:(g[0] + ng_) * P]
                        nc.vector.tensor_tensor(
                            out=sl_.rearrange("p (k c) -> p k c", c=P),
                            in0=sl_.rearrange("p (k c) -> p k c", c=P),
                            in1=urep[(l - 1) % 2][:, 0:P].unsqueeze(
                                1).broadcast_to([P, ng_, P]),
                            op=mybir.AluOpType.max)

                    psum = pagg.tile([P, gw], F32, name="psum", tag="psum",
                                     padded_shape=[P, 4 * P], space="PSUM")
                    # one PSUM accumulation group: self matmuls first (start
                    # on the very first), then per-chunk window-segment
                    # matmuls, stop on the last segment.
                    toff = 0
                    for ti, t in enumerate(g):
                        ts_ = cfg.tsize(t)
                        nc.tensor.matmul(
                            out=psum[:, toff:toff + ts_],
                            lhsT=selfbuf[0:ts_, t * P:t * P + P],
                            rhs=ident16[0:ts_, 0:ts_],
                            start=(ti == 0),
                            stop=((self_only or gather_only) and ti == len(g) - 1))
                        toff += ts_
                    if not (self_only or gather_only):
                        if l > 0:
                            # (-u) x (-(deg+1)) = u*(deg+1), the deferred +u
                            nc.tensor.matmul(
                                out=psum[:, 0:gw],
                                lhsT=urow[(l - 1) % 2][0:1, 0:P],
                                rhs=degn_sb[0:1, goff:goff + gw],
                                start=False, stop=False)
                        chunks = sched.group_chunks[gi]
                        nmm = sum(len(c_) for c_ in chunks)
                        s0 = chunks[0][0][1]      # first seg col of group
                        # one batched is_equal builds every one-hot of the
                        # group: oh_all[p, s, c] = (iota[c] == dstl[p, s0+s])
                        oh_all = ohp.tile([P, nmm * P], F16, name="oh", tag="oh")
                        nc.vector.tensor_tensor(
                            out=oh_all[:, :].rearrange("p (s c) -> p s c", c=P),
                            in0=iota16[:, 0:P].unsqueeze(1).broadcast_to(
                                [P, nmm, P]),
                            in1=dstl16[:, s0:s0 + nmm].unsqueeze(2).broadcast_to(
                                [P, nmm, P]),
                            op=mybir.AluOpType.is_equal)
                        mm = 0
                        for j, segs in enumerate(chunks):
                            for (w, scol) in segs:
                                ts_ = cfg.tsize(w)
                                woff = (w - g[0]) * P
                                so = (scol - s0) * P
                                mm += 1
                                nc.tensor.matmul(
                                    out=psum[:, woff:woff + ts_],
                                    lhsT=gt[:, j * P:(j + 1) * P],
                                    rhs=oh_all[:, so:so + ts_],
                                    start=False, stop=(mm == nmm))

                    # ---- MLP ----
                    zt = up.tile([P, gw], F32, name="zt", tag="zt",
                                 padded_shape=[P, 4 * P])
                    zscale = 1.0 if l == 0 else scol_hist[:, l - 1:l]
                    nc.scalar.activation(out=zt[:, :], in_=psum[:, :],
                                         func=copy_op, bias=0.0, scale=zscale)
                    u_t = [up.tile([P, gw], F32, name=f"u{hh}", tag=f"u{hh}",
                                   padded_shape=[P, 4 * P]) for hh in range(2)]
                    for hh in range(2):
                        ps1 = pmlp.tile([P, gw], F32, name="ps1", tag="ps1",
                                        padded_shape=[P, 4 * P], space="PSUM")
                        nc.tensor.matmul(
                            out=ps1[:, :],
                            lhsT=w1_sb[:, l * 2 * P + hh * P:l * 2 * P + hh * P + P],
                            rhs=zt[:, :],
                            start=True, stop=True)
                        nc.scalar.activation(
                            out=u_t[hh][:, :], in_=ps1[:, :], func=relu_op,
                            bias=b1_sb[:, l * 2 + hh:l * 2 + hh + 1], scale=1.0)
                    ps2 = pmlp.tile([P, gw], F32, name="ps2", tag="ps2",
                                    padded_shape=[P, 4 * P], space="PSUM")
                    for hh in range(2):
                        nc.tensor.matmul(
                            out=ps2[:, :],
                            lhsT=w2_sb[:, (l * 2 + hh) * P:(l * 2 + hh) * P + P],
                            rhs=u_t[hh][:, :],
                            start=(hh == 0), stop=(hh == 1))
                    nc.vector.tensor_scalar(
                        out=z2all[:, goff:goff + gw], in0=ps2[:, :],
                        scalar1=b2_sb[:, l:l + 1], scalar2=None,
                        op0=mybir.AluOpType.add)
                    nc.vector.tensor_reduce(
                        out=ssum[:, gi:gi + 1], in_=z2all[:, goff:goff + gw],
                        axis=mybir.AxisListType.X, op=mybir.AluOpType.add)
                    sq_scr = scrp.tile([P, 4 * P], F32, name="sq_scr", tag="sq")
                    nc.scalar.activation(
                        out=sq_scr[:, 0:gw], in_=z2all[:, goff:goff + gw],
                        func=mybir.ActivationFunctionType.Square,
                        bias=zero_sb[:, 0:1],
                        accum_out=ssq[:, gi:gi + 1])

                    if last and not no_segmax:
                        # masked per-(graph x tile) maxes of raw z + BIG (>0),
                        # before BN stats arrive: max commutes with the final
                        # positive-scale affine normalize, applied later to
                        # the 128-col slot-max matrix instead of all nodes.
                        for t in g:
                            ts_ = cfg.tsize(t)
                            S = sched.gm_S[t]
                            o = int(gm_lo_off[t])
                            sb0 = int(gm_slot_base[t])
                            yb = mkp.tile([P, P], F32, name="yb", tag="yb")
                            if ts_ < P:
                                nc.vector.memset(yb[:, ts_:P], 0.0)
                            nc.scalar.activation(
                                out=yb[:, 0:ts_],
                                in_=z2all[:, t * P:t * P + ts_],
                                func=ident_op, bias=big_sb[:, 0:1], scale=1.0)
                            ge = mkp.tile([P, (S + 1) * P], F32, name="ge",
                                          tag="ge",
                                          padded_shape=[P, (SMAX + 1) * P])
                            nc.vector.tensor_tensor(
                                out=ge[:, :(S + 1) * P].rearrange(
                                    "p (s c) -> p s c", c=P),
                                in0=iota_sb[:, 0:P].unsqueeze(1).broadcast_to(
                                    [P, S + 1, P]),
                                in1=gmlo_sb[:, o:o + S + 1].unsqueeze(
                                    2).broadcast_to([P, S + 1, P]),
                                op=mybir.AluOpType.is_ge)
                            inr = mkp.tile([P, S * P], F32, name="inr",
                                           tag="inr",
                                           padded_shape=[P, SMAX * P])
                            nc.vector.tensor_tensor(
                                out=inr[:, :S * P], in0=ge[:, 0:S * P],
                                in1=ge[:, P:(S + 1) * P],
                                op=mybir.AluOpType.subtract)
                            # prod reuses ge's buffer (ge is consumed)
                            nc.vector.tensor_tensor(
                                out=ge[:, :S * P].rearrange(
                                    "p (s c) -> p s c", c=P),
                                in0=yb[:, 0:P].unsqueeze(1).broadcast_to(
                                    [P, S, P]),
                                in1=inr[:, :S * P].rearrange(
                                    "p (s c) -> p s c", c=P),
                                op=mybir.AluOpType.mult)
                            nc.vector.tensor_reduce(
                                out=gmM[:, sb0:sb0 + S],
                                in_=ge[:, :S * P].rearrange(
                                    "p (s c) -> p s c", c=P),
                                axis=mybir.AxisListType.X,
                                op=mybir.AluOpType.max)

                    if not last:
                        # transpose raw z tiles to row-major as each group's
                        # MLP lands; stage-0 AllGather fires mid-layer (after
                        # group 6 covers H0) so next-layer lo gathers overlap
                        # this layer's tail. Normalize is deferred (clip + u
                        # rank-1 + s fold) to the consumers next layer.
                        hout = hrm[l % 2]
                        for t in g:
                            ts_ = cfg.tsize(t)
                            zn = znp.tile([P, P], F16, name="zn16", tag="zn16")
                            nc.scalar.activation(out=zn[:, 0:ts_],
                                                 in_=z2all[:, t * P:t * P + ts_],
                                                 func=copy_op, bias=0.0,
                                                 scale=1.0)
                            tp = ptp.tile([P, P], F16, name="tp", tag="tp",
                                          space="PSUM")
                            nc.tensor.transpose(out=tp[0:ts_, :],
                                                in_=zn[:, 0:ts_],
                                                identity=ident16[:, :])
                            nc.vector.tensor_copy(
                                out=hout[0:ts_, t * P:t * P + P],
                                in_=tp[0:ts_, :])
                        if g[0] <= H0 // P - 1 <= g[-1]:
                            nc.sync.dma_start(
                                out=ag_in0[l][0:H0, :].rearrange(
                                    "(t p) f -> p t f", p=P),
                                in_=hout[:, 0:H0].rearrange(
                                    "p (t f) -> p t f", f=P))
                            if not no_ag:
                                nc.gpsimd.collective_compute(
                                    "AllGather", mybir.AluOpType.bypass,
                                    replica_groups=rg,
                                    ins=[ag_in0[l][:, :]],
                                    outs=[ag_out0[l][:, :]])
                        if gi == len(cfg.groups) - 1:
                            n1full = H1 // P
                            nc.sync.dma_start(
                                out=ag_in1[l][0:n1full * P, :].rearrange(
                                    "(t p) f -> p t f", p=P),
                                in_=hout[:, H0:H0 + n1full * P].rearrange(
                                    "p (t f) -> p t f", f=P))
                            if H1 % P:
                                ts_ = H1 % P
                                nc.sync.dma_start(
                                    out=ag_in1[l][n1full * P:H1, :],
                                    in_=hout[0:ts_,
                                             H0 + n1full * P:H0 + n1full * P + P])

                # ---- BN stats allreduce (issued before AG#1 so the tiny
                # latency-critical AR isn't queued behind the bulk AG on the
                # collective cores; urep gates the next layer's first clip) ----
                ar_sb = sp.tile([P, 2], F32, name="ar_sb", tag="ar")
                nc.vector.tensor_reduce(out=ar_sb[:, 0:1], in_=ssum[:, :],
                                        axis=mybir.AxisListType.X,
                                        op=mybir.AluOpType.add)
                nc.vector.tensor_reduce(out=ar_sb[:, 1:2], in_=ssq[:, :],
                                        axis=mybir.AxisListType.X,
                                        op=mybir.AluOpType.add)
                if no_ar:
                    arr = ar_sb
                else:
                    nc.sync.dma_start(out=ar_in[l][:, :], in_=ar_sb[:, :])
                    nc.gpsimd.collective_compute(
                        "AllReduce", mybir.AluOpType.add, replica_groups=rg,
                        ins=[ar_in[l][:, :]], outs=[ar_out[l][:, :]])
                    arr = sp.tile([P, 2], F32, name="arr", tag="ar")
                    nc.sync.dma_start(out=arr[:, :], in_=ar_out[l][:, :])
                if not last and not no_ag:
                    nc.gpsimd.collective_compute(
                        "AllGather", mybir.AluOpType.bypass, replica_groups=rg,
                        ins=[ag_in1[l][:, :]], outs=[ag_out1[l][:, :]])

                stat = sp.tile([P, 6], F32, name="stat", tag="stat")
                mean, msq, var, istd, s_col, t_col = [stat[:, i:i + 1] for i in range(6)]
                nc.vector.tensor_scalar(out=mean, in0=arr[:, 0:1], scalar1=inv_n,
                                        scalar2=None, op0=mybir.AluOpType.mult)
                nc.vector.tensor_scalar(out=msq, in0=arr[:, 1:2], scalar1=inv_n,
                                        scalar2=None, op0=mybir.AluOpType.mult)
                sq_t = sp.tile([P, 2], F32, name="sq_t", tag="sq_t")
                nc.vector.tensor_tensor(out=sq_t[:, 0:1], in0=mean, in1=mean,
                                        op=mybir.AluOpType.mult)
                nc.vector.tensor_tensor(out=var, in0=msq, in1=sq_t[:, 0:1],
                                        op=mybir.AluOpType.subtract)
                std_t = sp.tile([P, 2], F32, name="std_t", tag="sq_t")
                nc.scalar.activation(out=std_t[:, 0:1], in_=var,
                                     func=mybir.ActivationFunctionType.Sqrt,
                                     bias=eps_sb[:, 0:1], scale=1.0)
                nc.vector.reciprocal(out=istd, in_=std_t[:, 0:1])
                nc.vector.tensor_tensor(out=s_col, in0=gam_sb[:, l:l + 1], in1=istd,
                                        op=mybir.AluOpType.mult)
                nc.vector.tensor_tensor(out=sq_t[:, 1:2], in0=mean, in1=s_col,
                                        op=mybir.AluOpType.mult)
                nc.vector.tensor_tensor(out=t_col, in0=bet_sb[:, l:l + 1],
                                        in1=sq_t[:, 1:2],
                                        op=mybir.AluOpType.subtract)
                if not last:
                    # deferred-normalize params for layer l+1: u = t/s as a
                    # row broadcast to all partitions, s per-feature column
                    nc.vector.tensor_copy(out=scol_hist[:, l:l + 1], in_=s_col)
                    uc = sp.tile([P, 2], F32, name="uc", tag="uc")
                    nc.vector.reciprocal(out=uc[:, 0:1], in_=s_col)
                    nc.vector.tensor_tensor(out=uc[:, 1:2], in0=t_col,
                                            in1=uc[:, 0:1],
                                            op=mybir.AluOpType.mult)
                    u16 = sp.tile([P, 1], F16, name="u16", tag="u16")
                    nc.vector.tensor_scalar(out=u16[:, 0:1], in0=uc[:, 1:2],
                                            scalar1=-1.0, scalar2=None,
                                            op0=mybir.AluOpType.mult)
                    urtp = ptp.tile([P, P], F16, name="urtp", tag="tp",
                                    space="PSUM")
                    nc.tensor.transpose(out=urtp[0:1, :], in_=u16[:, 0:1],
                                        identity=ident16[:, :])
                    nc.vector.tensor_copy(out=urow[l % 2][0:1, :],
                                          in_=urtp[0:1, :])
                    # broadcast -u to all partitions via PE (ones x u_row);
                    # partition_broadcast would stall the in-order Pool queue
                    # on the AR result and delay next-layer gather desc-gen
                    ub_ps = pagg.tile([P, P], F32, name="ubps", tag="psum",
                                      padded_shape=[P, 4 * P], space="PSUM")
                    nc.tensor.matmul(out=ub_ps[:, :],
                                     lhsT=ones_row[0:1, 0:P],
                                     rhs=urow[l % 2][0:1, :],
                                     start=True, stop=True)
                    nc.vector.tensor_copy(out=urep[l % 2][:, :],
                                          in_=ub_ps[:, :])

                if last and no_segmax:
                    gz = cp.tile([G, P], F32, name="gz")
                    nc.vector.memset(gz[:, :], 0.0)
                    nc.sync.dma_start(out=gmax_out[:, :], in_=gz[:, :])
                else:
                    # ---- finish global max pool: normalize the slot-max
                    # matrix (max commutes with the positive-scale affine),
                    # select each graph's k-th slot via one-hot matmuls,
                    # max-combine, AllReduce-max across cores.
                    stmp = cp.tile([P, 2], F32, name="gmt")
                    nc.vector.tensor_scalar(out=stmp[:, 0:1], in0=s_col,
                                            scalar1=GM_BIG, scalar2=None,
                                            op0=mybir.AluOpType.mult)
                    nc.vector.tensor_tensor(out=stmp[:, 1:2], in0=t_col,
                                            in1=stmp[:, 0:1],
                                            op=mybir.AluOpType.subtract)
                    gmMn = cp.tile([P, P], F32, name="gmMn")
                    nc.scalar.activation(out=gmMn[:, :], in_=gmM[:, :],
                                         func=ident_op, bias=stmp[:, 1:2],
                                         scale=s_col)
                    nc.vector.memset(gmMn[:, 127:128], GM_SENT)
                    tpg = pagg.tile([P, P], F32, name="gmtp", tag="psum",
                                    padded_shape=[P, 4 * P], space="PSUM")
                    nc.tensor.transpose(out=tpg[:, :], in_=gmMn[:, :],
                                        identity=ident_sb[:, :])
                    gmMT = cp.tile([P, P], F32, name="gmMT")
                    nc.vector.tensor_copy(out=gmMT[:, :], in_=tpg[:, :])
                    gcur = [cp.tile([G, P], F32, name=f"gc{i}")
                            for i in range(2)]
                    for k in range(GMK):
                        psg = pmlp.tile([G, P], F32, name="gmps", tag="ps1",
                                        padded_shape=[P, 4 * P],
                                        space="PSUM")
                        nc.tensor.matmul(out=psg[:, :],
                                         lhsT=gmsel_sb[:, k * G:(k + 1) * G],
                                         rhs=gmMT[:, :],
                                         start=True, stop=True)
                        if k == 0:
                            nc.vector.tensor_copy(out=gcur[0][:, :],
                                                  in_=psg[:, :])
                        else:
                            nc.vector.tensor_tensor(
                                out=gcur[k % 2][:, :],
                                in0=gcur[(k + 1) % 2][:, :], in1=psg[:, :],
                                op=mybir.AluOpType.max)
                    gfin = gcur[(GMK - 1) % 2]
                    if no_ar:
                        nc.sync.dma_start(out=gmax_out[:, :], in_=gfin[:, :])
                    else:
                        nc.sync.dma_start(out=gm_in[:, :], in_=gfin[:, :])
                        nc.gpsimd.collective_compute(
                            "AllReduce", mybir.AluOpType.max,
                            replica_groups=rg,
                            ins=[gm_in[:, :]], outs=[gm_out[:, :]])
                        gmax2 = cp.tile([G, P], F32, name="gmax2")
                        nc.sync.dma_start(out=gmax2[:, :], in_=gm_out[:, :])
                        nc.sync.dma_start(out=gmax_out[:, :], in_=gmax2[:, :])

    nc.compile()
    return nc


def prep_inputs(cfg: Cfg, sched: Sched, x, W1, b1, W2, b2, gamma, beta):
    """Per-core input maps. Host does data layout only: transpose/cast/shard."""
    N, L, ntiles, npc = cfg.N, cfg.L, cfg.ntiles, cfg.npc
    x = np.asarray(x, np.float32)
    x16 = np.ascontiguousarray(x.astype(np.float16))
    H0 = 28 * P
    xr = np.ascontiguousarray(np.concatenate(
        [x16[c * npc:c * npc + H0] for c in range(NC)] +
        [x16[c * npc + H0:(c + 1) * npc] for c in range(NC)], axis=0))
    iota = np.broadcast_to(np.arange(P, dtype=np.float32), (P, P)).copy()
    ident = np.eye(P, dtype=np.float32)
    w1 = np.ascontiguousarray(np.transpose(np.asarray(W1, np.float32), (1, 0, 2))
                              ).reshape(P, L * 2 * P)
    w2 = np.ascontiguousarray(np.transpose(
        np.asarray(W2, np.float32).reshape(L, 2, P, P), (2, 0, 1, 3))
        ).reshape(P, L * 2 * P)
    b1r = np.ascontiguousarray(np.transpose(
        np.asarray(b1, np.float32).reshape(L, 2, P), (2, 0, 1))).reshape(P, L * 2)
    b2r = np.ascontiguousarray(np.asarray(b2, np.float32).T)
    gam = np.ascontiguousarray(np.asarray(gamma, np.float32).T)
    bet = np.ascontiguousarray(np.asarray(beta, np.float32).T)

    in_maps = []
    for c in range(NC):
        shard = x16[c * npc:(c + 1) * npc]
        xh = np.zeros((P, ntiles * P), np.float16)
        nf = npc // P
        xh[:, :nf * P] = shard[:nf * P].reshape(nf, P, P).transpose(1, 0, 2).reshape(P, nf * P)
        if npc % P:
            xh[0:npc % P, nf * P:(nf + 1) * P] = shard[nf * P:]
        in_maps.append({
            "x_hrm": xh, "x_tab": xr,
            "idx16": sched.idx16[c], "gmlo": sched.gm_lo[c],
            "gmsel": sched.gm_sel[c], "degn": sched.degn[c],
            "dstl": sched.dstl[c],
            "iota": iota, "ident": ident,
            "w1": w1, "w2": w2, "b1": b1r, "b2": b2r, "gam": gam, "bet": bet,
        })
    return in_maps


# ---------------------------------------------------------------------------
# PJRT runner: compile once, stage inputs on device, reuse the executable
# ---------------------------------------------------------------------------

def make_runner(nc, n_cores=NC):
    """Build a reusable jitted executable for the Bass module (axon/PJRT)."""
    import jax
    import numpy as _np
    from jax.sharding import Mesh, PartitionSpec
    from jax.experimental.shard_map import shard_map
    import concourse.bass2jax as b2j

    b2j.install_neuronx_cc_hook()
    partition_name = nc.partition_id_tensor.name if nc.partition_id_tensor else None
    in_names, out_names, out_avals, zero_shapes = [], [], [], []
    for alloc in nc.m.functions[0].allocations:
        if not isinstance(alloc, mybir.MemoryLocationSet):
            continue
        name = alloc.memorylocations[0].name
        if alloc.kind == "ExternalInput":
            if name != partition_name:
                in_names.append(name)
        elif alloc.kind == "ExternalOutput":
            out_names.append(name)
            shape = tuple(alloc.tensor_shape)
            dtype = mybir.dt.np(alloc.dtype)
            out_avals.append(jax.core.ShapedArray(shape, dtype))
            zero_shapes.append((shape, dtype))
    n_params = len(in_names)
    all_in = list(in_names) + list(out_names)
    if partition_name is not None:
        all_in.append(partition_name)

    def _body(*args):
        operands = list(args)
        if partition_name is not None:
            operands.append(b2j.partition_id_tensor())
        outs = b2j._bass_exec_p.bind(
            *operands,
            out_avals=tuple(out_avals),
            in_names=tuple(all_in),
            out_names=tuple(out_names),
            lowering_input_output_aliases=(),
            sim_require_finite=True,
            sim_require_nnan=True,
            nc=nc,
        )
        return tuple(outs)

    devices = jax.devices()[:n_cores]
    mesh = Mesh(_np.asarray(devices), ("core",))
    donate = tuple(range(n_params, n_params + len(out_names)))
    in_specs = (PartitionSpec("core"),) * (n_params + len(out_names))
    out_specs = (PartitionSpec("core"),) * len(out_names)
    sharded = jax.jit(
        shard_map(_body, mesh=mesh, in_specs=in_specs, out_specs=out_specs,
                  check_rep=False),
        donate_argnums=donate, keep_unused=True)
    return sharded, in_names, out_names, zero_shapes, mesh


_CACHE = {}


def _get_compiled(cfg, edge_index, batch):
    key = (cfg.N, cfg.E, cfg.L, cfg.G,
           hashlib.blake2b(np.ascontiguousarray(edge_index).tobytes(),
                           digest_size=16).hexdigest(),
           hashlib.blake2b(np.ascontiguousarray(batch).tobytes(),
                           digest_size=16).hexdigest())
    if key not in _CACHE:
        sched = build_schedule(cfg, edge_index, batch)
        nc = build_nc(cfg, sched)
        runner = make_runner(nc, NC)
        _CACHE[key] = (sched, nc, runner)
    return _CACHE[key]


def kernel(x, edge_index, batch, num_graphs, W1, b1, W2, b2, gamma, beta):
    """GIN forward on 8 TRN2 NeuronCores. Full inputs in, full output out."""
    import jax
    from jax.sharding import NamedSharding, PartitionSpec

    x = np.asarray(x, np.float32)
    edge_index = np.asarray(edge_index)
    batch = np.asarray(batch)
    G = int(np.asarray(num_graphs))
    cfg = Cfg(N=x.shape[0], E=edge_index.shape[1], L=np.asarray(W1).shape[0], G=G)

    sched, nc, (sharded, in_names, out_names, zero_shapes, mesh) = \
        _get_compiled(cfg, edge_index, batch)

    in_maps = prep_inputs(cfg, sched, x, W1, b1, W2, b2, gamma, beta)
    sh = NamedSharding(mesh, PartitionSpec("core"))
    concat_in = [np.concatenate([np.asarray(in_maps[c][n]) for c in range(NC)],
                                axis=0) for n in in_names]
    dev_in = [jax.device_put(a, sh) for a in concat_in]
    zeros = [jax.device_put(np.zeros((NC * s[0], *s[1:]), d), sh)
             for s, d in zero_shapes]
    outs = sharded(*dev_in, *zeros)
    gmaxT = np.asarray(outs[out_names.index("gmaxT")])  # [NC*G, P]
    out = np.ascontiguousarray(gmaxT[:G].astype(np.float32))
    # match jax segment_max: empty segments are -inf (sentinel -30000)
    out[out <= -20000.0] = -np.inf
    return out

